# revision 104
# baseline (speedup 1.0000x reference)
"""Trainium2 Bass kernel for nn_CopulaDecoder — v3 (unified pipeline).

Sharding: data-parallel over batch B=8 -> 8 NeuronCores.

Structure (cost-model 870us vs 1052us for the v2 phase-sequential design):
  - phase 1 (~160us, PE-bound ~99%): q0 = enc@Wds interleaved per-chunk with
    the layer-0 K/V MLP (q0 emitted right after each XT chunk transpose so PE
    is never head-of-line blocked on staging); W3+kv-staging lagged one chunk;
    relus split Act/DVE; NLL mask precompute on Pool; q-gather + av init
    overlap the MLP tail; first two attention gathers issue as soon as kv0
    staging lands.
  - unified 32-step loop (layer-0 tiles then layer-1 tiles, ~DVE 90-95%):
    per step: attention score half (DVE) -> deferred LN of the PREVIOUS tile
    (hides the Act exp round-trip) -> value half (DVE + Pool n-row slice that
    adds its partial straight to av, never touching Es, so the next tile's
    score path reuses Es without waiting on Pool); ff block + decoder lag two
    steps; layer-1 K/V MLP spread over the first 14 steps (relus on Act only:
    GPSIMD cannot read PSUM); gathers prefetched depth-2 across the layer
    boundary with a third G buffer opened mid-loop from SBUF freed by the
    right-side weight pool.
  - bf16 residual stream (av): halves SBUF, drops the qt cast copy (score
    mult reads av directly), LN apply runs in DVE 4x mode.
  - host passes enc/W1/W2/W3/Wds/fW1/fW2 pre-cast to bf16 (gpsimd cast DMAs
    were serializing the head); W3 bias via one whole-bank ones-row matmul.

Numerics: all big attention elementwise ops in DVE 2x mode ((d,h)-packed
bf16); kv rows in DRAM are [k(d,h) | v(d,h)]; no softmax max-subtraction
(scores are tiny); LN rstd via Newton on [128,1]; rel err ~4e-3 vs fp32 ref.
"""
import sys

sys.path.insert(0, "/opt/trn_rl_repo")

import math
import ml_dtypes
import numpy as np

BF16NP = ml_dtypes.bfloat16

import concourse.bacc as bacc
import concourse.bass as bass
import concourse.bass_isa as bass_isa
import concourse.mybir as mybir
import concourse.tile as tile
from concourse.bass_utils import run_bass_kernel_spmd
from concourse.masks import make_identity

F32 = mybir.dt.float32
BF16 = mybir.dt.bfloat16
I16 = mybir.dt.int16
I32T = mybir.dt.int32

B, V, P, N = 8, 4096, 2048, 32
IN_DIM, H, D, L = 256, 8, 32, 2
MLP, RES = 128, 100
HD = H * D
EPS = 1e-5
SCALE = D ** -0.5
LOG_RES = math.log(RES)

KVROW = 2 * HD
NT = P // 128
VT = V // 128
TCH = 512
ICH = 128 * N // 16  # idx cols per point-tile in wrapped layout


def _wrap_idx(idx_flat):
    n = idx_flat.shape[0]
    w = idx_flat.reshape(n // 16, 16).T.astype(np.int16)
    return np.tile(w, (8, 1)).copy()


def build_program(ln_trivial):
    nc = bacc.Bacc()
    op = mybir.AluOpType
    ACTF = mybir.ActivationFunctionType
    X = mybir.AxisListType.X

    # ---------------- DRAM tensors ----------------
    enc = nc.dram_tensor("enc", [V, IN_DIM], BF16, kind="ExternalInput")
    uu = nc.dram_tensor("uu", [V, 1], F32, kind="ExternalInput")
    # per-head MLP weights, batched layouts (see host prep):
    # w1a: [128, L*S*H*C, MLP] rows (c p) of W1[:256]; w1u: [1, L*S*H, MLP]
    w1a = nc.dram_tensor("w1a", [128, L * 2 * H * 2 * MLP], BF16, kind="ExternalInput")
    w1u = nc.dram_tensor("w1u", [1, L * 2 * H * MLP], BF16, kind="ExternalInput")
    w2a = nc.dram_tensor("w2a", [128, L * 2 * H * MLP], BF16, kind="ExternalInput")
    w3a = nc.dram_tensor("w3a", [128, L * 2 * H * D], BF16, kind="ExternalInput")
    w1uc = nc.dram_tensor("w1uc", [128, L * 2 * H], F32, kind="ExternalInput")
    b1a = nc.dram_tensor("b1a", [128, L * 2 * H], F32, kind="ExternalInput")
    b2a = nc.dram_tensor("b2a", [128, L * 2 * H], F32, kind="ExternalInput")
    b3shd = nc.dram_tensor("b3shd", [L, KVROW], F32, kind="ExternalInput")  # (s,h,d)
    Wds = nc.dram_tensor("Wds", [IN_DIM, HD], BF16, kind="ExternalInput")  # cols perm
    bds = nc.dram_tensor("bds", [1, HD], F32, kind="ExternalInput")       # perm
    fW1 = nc.dram_tensor("fW1", [L, HD, HD], BF16, kind="ExternalInput")   # both perm
    fb1 = nc.dram_tensor("fb1", [L, HD], F32, kind="ExternalInput")
    fW2 = nc.dram_tensor("fW2", [L, HD, HD], BF16, kind="ExternalInput")
    fb2 = nc.dram_tensor("fb2", [L, HD], F32, kind="ExternalInput")
    dW1 = nc.dram_tensor("dW1", [HD, MLP], F32, kind="ExternalInput")     # rows perm
    db1 = nc.dram_tensor("db1", [MLP, 1], F32, kind="ExternalInput")
    dW2 = nc.dram_tensor("dW2", [MLP, MLP], F32, kind="ExternalInput")
    db2 = nc.dram_tensor("db2", [MLP, 1], F32, kind="ExternalInput")
    dW3 = nc.dram_tensor("dW3", [MLP, RES], F32, kind="ExternalInput")
    db3 = nc.dram_tensor("db3", [1, RES], F32, kind="ExternalInput")
    ln_g = nc.dram_tensor("ln_g", [2 * L, HD], F32, kind="ExternalInput")  # perm
    ln_b = nc.dram_tensor("ln_b", [2 * L, HD], F32, kind="ExternalInput")
    att_idx = nc.dram_tensor("att_idx", [128, NT * ICH], I16, kind="ExternalInput")
    pred_idx = nc.dram_tensor("pred_idx", [128, P // 16], I16, kind="ExternalInput")
    u_pred = nc.dram_tensor("u_pred", [128, NT], F32, kind="ExternalInput")

    loss_out = nc.dram_tensor("loss_out", [1, 1], F32, kind="ExternalOutput")

    q0d = nc.dram_tensor("q0d", [V, HD], BF16)
    kvd = [nc.dram_tensor(f"kv{l}", [V, KVROW], BF16) for l in range(L)]

    with tile.TileContext(nc) as tc:
        wc_cm = tc.tile_pool(name="wc", bufs=1)
        wc = wc_cm.__enter__()
        # pool for tiles only needed through pass A (freed for pass B)
        pxt_cm = tc.tile_pool(name="pxt", bufs=1, side="right")
        pxt = pxt_cm.__enter__()

        # ---------------- persistent small tiles ----------------
        ident = wc.tile([128, 128], F32)
        make_identity(nc, ident[:])
        identb = wc.tile([128, 128], BF16)
        make_identity(nc, identb[:])
        eps_t = wc.tile([128, 1], F32)
        nc.vector.memset(eps_t[:], EPS)
        ones_row = wc.tile([1, 128], BF16)
        nc.vector.memset(ones_row[:], 1.0)
        iota100 = wc.tile([128, RES], BF16)
        nc.gpsimd.iota(iota100[:], [[1, RES]], channel_multiplier=0,
                       allow_small_or_imprecise_dtypes=True)

        # ---------------- batched weight loads ----------------
        # per-head MLP weights: tile [128, l, s, h, c, m] etc.
        W1 = pxt.tile([128, L, 2, H, 2, MLP], BF16)
        nc.gpsimd.dma_start(
            out=W1[:], in_=w1a[:, :].rearrange(
                "p (l s h c m) -> p l s h c m", l=L, s=2, h=H, c=2))
        W1U = pxt.tile([1, L, 2, H, MLP], BF16)
        nc.gpsimd.dma_start(
            out=W1U[:], in_=w1u[:, :].rearrange("p (l s h m) -> p l s h m", l=L, s=2, h=H))
        W2 = pxt.tile([128, L, 2, H, MLP], BF16)
        nc.gpsimd.dma_start(
            out=W2[:], in_=w2a[:, :].rearrange("p (l s h m) -> p l s h m", l=L, s=2, h=H))
        W3 = pxt.tile([128, L, 2, H, D], BF16)
        nc.gpsimd.dma_start(
            out=W3[:], in_=w3a[:, :].rearrange("p (l s h m) -> p l s h m", l=L, s=2, h=H))
        B1 = pxt.tile([128, L, 2, H], F32)
        nc.sync.dma_start(
            out=B1[:], in_=b1a[:, :].rearrange("p (l s h) -> p l s h", l=L, s=2))
        B2 = pxt.tile([128, L, 2, H], F32)
        nc.sync.dma_start(
            out=B2[:], in_=b2a[:, :].rearrange("p (l s h) -> p l s h", l=L, s=2))
        B3row = []
        for l in range(L):
            t = pxt.tile([1, KVROW], BF16, tag=f"B3row{l}")
            nc.gpsimd.dma_start(out=t[:], in_=b3shd[l:l + 1, :])
            B3row.append(t)

        WdsC = pxt.tile([128, 2, HD], BF16)
        nc.gpsimd.dma_start(out=WdsC[:], in_=Wds[:, :].rearrange("(c p) m -> p c m", p=128))
        BdsT = pxt.tile([128, HD], F32)
        nc.sync.dma_start(out=BdsT[:], in_=bds[0:1, :].broadcast_to([128, HD]))

        # ---------------- staging: merged bf16 + X^T (per-chunk so the
        # MLP can start on chunk 0 while later chunks stage) ----------------
        XT = pxt.tile([128, 2, V], BF16)
        for c8 in range(V // TCH):
            r0 = c8 * TCH
            for c in range(2):
                nc.sync.dma_start_transpose(
                    out=XT[:, c, r0:r0 + TCH],
                    in_=enc[r0:r0 + TCH, c * 128:(c + 1) * 128])
        XU = pxt.tile([1, V], BF16)
        nc.gpsimd.dma_start(out=XU[:], in_=uu[:, :].rearrange("v e -> e v"))
        W1UC = pxt.tile([128, L, 2, H], F32)
        nc.sync.dma_start(
            out=W1UC[:], in_=w1uc[:, :].rearrange("p (l s h) -> p l s h",
                                                  l=L, s=2))

        FW1 = wc.tile([128, L, 2, HD], BF16)
        nc.gpsimd.dma_start(out=FW1[:], in_=fW1[:, :, :].rearrange(
            "l (c p) m -> p l c m", p=128))
        FW2 = wc.tile([128, L, 2, HD], BF16)
        nc.gpsimd.dma_start(out=FW2[:], in_=fW2[:, :, :].rearrange(
            "l (c p) m -> p l c m", p=128))
        FB1 = wc.tile([128, L, 2], F32)
        nc.sync.dma_start(out=FB1[:], in_=fb1[:, :].rearrange("l (c p) -> p l c", p=128))
        FB2 = wc.tile([128, L, 2], F32)
        nc.sync.dma_start(out=FB2[:], in_=fb2[:, :].rearrange("l (c p) -> p l c", p=128))
        LNG, LNB = [], []
        if not ln_trivial:
            for i in range(2 * L):
                t = wc.tile([128, HD], F32, tag=f"LNG{i}")
                nc.sync.dma_start(out=t[:], in_=ln_g[i:i + 1, :].broadcast_to([128, HD]))
                LNG.append(t)
                t = wc.tile([128, HD], F32, tag=f"LNB{i}")
                nc.sync.dma_start(out=t[:], in_=ln_b[i:i + 1, :].broadcast_to([128, HD]))
                LNB.append(t)

        DW1 = wc.tile([128, 2, MLP], BF16)
        nc.gpsimd.dma_start(out=DW1[:], in_=dW1[:, :].rearrange("(c p) m -> p c m", p=128))
        DB1 = wc.tile([128, 1], F32)
        nc.sync.dma_start(out=DB1[:], in_=db1[:, :])
        DW2 = wc.tile([128, MLP], BF16)
        nc.gpsimd.dma_start(out=DW2[:], in_=dW2[:, :])
        DB2 = wc.tile([128, 1], F32)
        nc.sync.dma_start(out=DB2[:], in_=db2[:, :])
        DW3 = wc.tile([128, RES], BF16)
        nc.gpsimd.dma_start(out=DW3[:], in_=dW3[:, :])
        DB3 = wc.tile([1, RES], BF16)
        nc.gpsimd.dma_start(out=DB3[:], in_=db3[:, :])

        idx_att = wc.tile([128, NT * ICH], I16)
        nc.sync.dma_start(out=idx_att[:], in_=att_idx[:, :])
        idx_pred = wc.tile([128, P // 16], I16)
        nc.sync.dma_start(out=idx_pred[:], in_=pred_idx[:, :])

        av = []
        for t in range(NT):
            av_t = wc.tile([128, HD], BF16, tag=f"av{t}")
            av.append(av_t)
        u_keep = wc.tile([128, NT], F32)
        nc.sync.dma_start(out=u_keep[:], in_=u_pred[:, :])
        loss_acc = wc.tile([128, 1], F32)
        nc.vector.memset(loss_acc[:], 0.0)

        MAGIC = 0x5F3759DF

        def ln_tile(pool, t, gi, rs=None):
            """Per-tile LN on av[t]: stats/apply on DVE, Newton rsqrt on
            `rs` (Pool when it has slack — all SBUF so legal there)."""
            rs = rs or nc.vector
            st = pool.tile([128, 6], F32, tag="ln_st")
            nc.vector.bn_stats(out=st[:], in_=av[t][:])
            mv = pool.tile([128, 2], F32, tag="ln_mv")
            nc.vector.bn_aggr(out=mv[:], in_=st[:])
            ve = pool.tile([128, 1], F32, tag="ln_ve")
            rs.tensor_scalar(out=ve[:], in0=mv[:, 1:2], scalar1=EPS,
                             scalar2=None, op0=op.add, op1=op.bypass)
            sh = pool.tile([128, 1], I32T, tag="ln_sh")
            rs.tensor_scalar(out=sh[:], in0=ve[:].bitcast(I32T),
                             scalar1=1, scalar2=None,
                             op0=op.arith_shift_right, op1=op.bypass)
            rs.tensor_scalar(out=sh[:], in0=sh[:], scalar1=-1,
                             scalar2=MAGIC, op0=op.mult, op1=op.add)
            yy = pool.tile([128, 1], F32, tag="ln_yy")
            rs.tensor_copy(out=yy[:], in_=sh[:].bitcast(F32))
            t1 = pool.tile([128, 1], F32, tag="ln_t1")
            for _ in range(2):
                rs.tensor_tensor(out=t1[:], in0=yy[:], in1=yy[:], op=op.mult)
                rs.tensor_tensor(out=t1[:], in0=t1[:], in1=ve[:], op=op.mult)
                rs.tensor_scalar(out=t1[:], in0=t1[:], scalar1=-0.5,
                                 scalar2=1.5, op0=op.mult, op1=op.add)
                rs.tensor_tensor(out=yy[:], in0=yy[:], in1=t1[:], op=op.mult)
            nc.vector.tensor_scalar(out=av[t][:], in0=av[t][:], scalar1=mv[:, 0:1],
                                    scalar2=yy[:], op0=op.subtract, op1=op.mult)
            if not ln_trivial:
                nc.vector.tensor_tensor(out=av[t][:], in0=av[t][:], in1=LNG[gi][:],
                                        op=op.mult)
                nc.vector.tensor_tensor(out=av[t][:], in0=av[t][:], in1=LNB[gi][:],
                                        op=op.add)

        # ---------------- K/V MLP chunk (split into parts) ----------------
        def mlp_part12(pm, pmh, ps1, ps2, h2s, l, tch, pairs, relu1s, relu2s,
                       depth=2, fold_u=False):
            """W1+W2 stages for `pairs` of the TCH-token chunk `tch`."""
            tsl = slice(tch * TCH, (tch + 1) * TCH)
            if fold_u:
                xub = pmh.tile([128, TCH], BF16, tag="xub")
                nc.gpsimd.dma_start(
                    out=xub[:],
                    in_=uu[tsl, :].rearrange("v e -> e v").broadcast_to(
                        [128, TCH]))

            def w1_stage(s, h):
                pm1 = ps1.tile([128, TCH], F32, tag="pm1")
                nc.tensor.matmul(pm1[:], W1[:, l, s, h, 0, :], XT[:, 0, tsl],
                                 start=True, stop=False)
                if fold_u and (s * H + h) % 2 == 0:
                    nc.tensor.matmul(pm1[:], W1[:, l, s, h, 1, :], XT[:, 1, tsl],
                                     start=False, stop=True)
                    # u-term as a rank-1 update into psum (saves a full PE
                    # column stream vs the K=1 matmul); alternate DVE/Pool
                    nc.vector.scalar_tensor_tensor(
                        out=pm1[:], in0=xub[:],
                        scalar=W1UC[:, l, s, h:h + 1], in1=pm1[:],
                        op0=op.mult, op1=op.add)
                else:
                    nc.tensor.matmul(pm1[:], W1[:, l, s, h, 1, :], XT[:, 1, tsl],
                                     start=False, stop=False)
                    nc.tensor.matmul(pm1[:], W1U[:, l, s, h, :], XU[0:1, tsl],
                                     start=False, stop=True)
                return pm1

            def w2_stage(s, h, pm1):
                h1 = pm.tile([128, TCH], BF16, tag="h1")
                relu1s[(s * H + h) % len(relu1s)](h1, pm1, B1[:, l, s, h:h + 1])
                pm2 = ps2.tile([128, TCH], F32, tag="pm2")
                nc.tensor.matmul(pm2[:], W2[:, l, s, h, :], h1[:],
                                 start=True, stop=True)
                h2 = pmh.tile([128, TCH], BF16, tag=f"h2_{s}_{h}")
                relu2s[(s * H + h) % len(relu2s)](h2, pm2, B2[:, l, s, h:h + 1])
                h2s[s, h] = h2

            pending = []
            for s, h in pairs:
                pending.append((s, h, w1_stage(s, h)))
                if len(pending) >= depth:
                    w2_stage(*pending.pop(0))
            for pr in pending:
                w2_stage(*pr)

        def mlp_part3(pm, ps3, h2s, l, tch):
            """W3 + (d,h)-permuted staging for chunk `tch` (needs all h2s)."""
            for m in range(TCH // 128):
                pkv = ps3.tile([128, 2, H, D], F32, tag="pkv")
                msl = slice(m * 128, (m + 1) * 128)
                # one whole-bank bias matmul, then per-(s,h) accumulation
                nc.tensor.matmul(
                    pkv[:].rearrange("p s h d -> p (s h d)"), ones_row[:],
                    B3row[l][:], start=True, stop=False)
                for i, (s, h) in enumerate([(s, h) for s in range(2)
                                            for h in range(H)]):
                    nc.tensor.matmul(
                        pkv[:, s, h, :], h2s[s, h][:, msl], W3[:, l, s, h, :],
                        start=False, stop=(i == 2 * H - 1), skip_group_check=True)
                stg = pm.tile([128, KVROW], BF16, tag="stg")
                nc.scalar.copy(
                    out=stg[:].rearrange("p (s d h) -> p s d h", s=2, h=H),
                    in_=pkv[:].rearrange("p s h d -> p s d h"))
                row0 = tch * TCH + m * 128
                nc.sync.dma_start(
                    out=kvd[l][row0:row0 + 128, :].rearrange("(c p) e -> p c e", p=128),
                    in_=stg[:].unsqueeze(1))

        def mlp_chunk(pm, pmh, ps1, ps2, ps3, l, tch, relu1s, relu2s, depth=2,
                      fold_u=False):
            h2s = {}
            pairs = [(s, h) for s in range(2) for h in range(H)]
            mlp_part12(pm, pmh, ps1, ps2, h2s, l, tch, pairs, relu1s, relu2s,
                       depth=depth, fold_u=fold_u)
            mlp_part3(pm, ps3, h2s, l, tch)

        def relu_act(o, i, b):
            nc.scalar.activation(out=o[:], in_=i[:], func=ACTF.Relu, bias=b, scale=1.0)

        def relu_dve(o, i, b):
            nc.vector.tensor_scalar(out=o[:], in0=i[:], scalar1=b, scalar2=0.0,
                                    op0=op.add, op1=op.max)

        def relu_pool(o, i, b):
            nc.gpsimd.tensor_scalar(out=o[:], in0=i[:], scalar1=b, scalar2=0.0,
                                    op0=op.add, op1=op.max)

        # ---------------- attention tile ----------------
        def att_gather(patg, l, t):
            G = patg.tile([128, N, KVROW], BF16, tag="G")
            nc.gpsimd.dma_gather(
                G[:], kvd[l][:, :], idx_att[:, t * ICH:(t + 1) * ICH],
                num_idxs=128 * N, num_idxs_reg=128 * N, elem_size=KVROW,
                single_packet=False)
            return G

        def att_score(pat, l, t, G):
            """Score half of an attention tile: q*k, d-tree, exp issue."""
            Gv = G[:].rearrange("p n (s d h) -> p n s d h", s=2, h=H)
            qt = av[t][:].rearrange("p (d h) -> p d h", h=H)
            Es = pat.tile([128, N, D, H], BF16, tag="Es")
            nc.vector.tensor_tensor(
                out=Es[:], in0=Gv[:, :, 0, :, :],
                in1=qt.unsqueeze(1).broadcast_to([128, N, D, H]), op=op.mult)
            # in-place d-halving tree: Es[:, :, 0:k, :] += Es[:, :, k:2k, :]
            for k in (16, 8, 4, 2):
                nc.vector.tensor_tensor(out=Es[:, :, 0:k, :], in0=Es[:, :, 0:k, :],
                                        in1=Es[:, :, k:2 * k, :], op=op.add)
            sc = pat.tile([128, N, H], BF16, tag="sc")
            nc.vector.tensor_tensor(out=sc[:], in0=Es[:, :, 0, :],
                                    in1=Es[:, :, 1, :], op=op.add)
            wE = pat.tile([128, N, H], BF16, tag="wE")
            nc.scalar.activation(out=wE[:], in_=sc[:], func=ACTF.Exp,
                                 bias=0.0, scale=SCALE)
            return Es, wE

        def att_value(pat, l, t, G, Es, wE, nsv=0):
            """Value half: softmax weights, weighted sum; Pool takes the
            last `nsv` n-rows and adds its partial to av directly."""
            NS = N - nsv
            Gv = G[:].rearrange("p n (s d h) -> p n s d h", s=2, h=H)
            den = pat.tile([128, H], F32, tag="den")
            nc.vector.tensor_reduce(out=den[:], in_=wE[:].rearrange("p n h -> p h n"),
                                    axis=X, op=op.add)
            rden = pat.tile([128, H], F32, tag="rden")
            nc.vector.reciprocal(out=rden[:], in_=den[:])
            wn = pat.tile([128, N, H], BF16, tag=f"wn{t % 2}")
            nc.vector.tensor_tensor(
                out=wn[:], in0=wE[:],
                in1=rden[:].unsqueeze(1).broadcast_to([128, N, H]), op=op.mult)
            if nsv:
                # Pool slice: value-path rows [NS, N), halving-tree reduce
                pvm = pat.tile([128, nsv, D, H], BF16, tag=f"pvm{t % 2}")
                nc.gpsimd.tensor_tensor(
                    out=pvm[:], in0=Gv[:, NS:, 1, :, :],
                    in1=wn[:, NS:, :].unsqueeze(2).broadcast_to(
                        [128, nsv, D, H]), op=op.mult)
                Pm = pvm[:].rearrange("p n d h -> p n (d h)")
                k = nsv // 2
                while k >= 1:
                    nc.gpsimd.tensor_tensor(out=Pm[:, 0:k, :], in0=Pm[:, 0:k, :],
                                            in1=Pm[:, k:2 * k, :], op=op.add)
                    k //= 2
            nc.vector.tensor_tensor(  # Ev reuses Es storage
                out=Es[:, 0:NS], in0=Gv[:, 0:NS, 1, :, :],
                in1=wn[:, 0:NS, :].unsqueeze(2).broadcast_to(
                    [128, NS, D, H]), op=op.mult)
            Ev = Es[:].rearrange("p n d h -> p n (d h)")
            # in-place n-halving tree over [0, NS)
            if NS > 16:
                nc.vector.tensor_tensor(out=Ev[:, 0:NS - 16, :],
                                        in0=Ev[:, 0:NS - 16, :],
                                        in1=Ev[:, 16:NS, :], op=op.add)
            for k in (8, 4, 2):
                nc.vector.tensor_tensor(out=Ev[:, 0:k, :], in0=Ev[:, 0:k, :],
                                        in1=Ev[:, k:2 * k, :], op=op.add)
            nc.vector.tensor_tensor(out=Ev[:, 0, :], in0=Ev[:, 0, :],
                                    in1=Ev[:, 1, :], op=op.add)
            # DVE and Pool add their partials to av independently (no
            # cross-engine read of Es, so the next tile's score path can
            # reuse it immediately)
            nc.vector.tensor_tensor(out=av[t][:], in0=Ev[:, 0, :],
                                    in1=av[t][:], op=op.add)
            if nsv:
                Pm0 = pvm[:].rearrange("p n d h -> p n (d h)")[:, 0, :]
                nc.gpsimd.tensor_tensor(out=av[t][:], in0=Pm0,
                                        in1=av[t][:], op=op.add)

        def att_tile(pat, l, t, G, nsv=0):
            Es, wE = att_score(pat, l, t, G)
            att_value(pat, l, t, G, Es, wE, nsv=nsv)

        # ---------------- per-tile FF block for layer l ----------------
        def ff_tile(pf, psf, l, t, dec_xT=None, radd=None):
            xT = pf.tile([128, 2, 128], BF16, tag="xT")
            for c in range(2):
                ptx = psf.tile([128, 128], BF16, tag="pfx")
                nc.tensor.transpose(ptx[:], av[t][:, c * 128:(c + 1) * 128],
                                    identb[:])
                nc.scalar.copy(out=xT[:, c, :], in_=ptx[:])
            hT = pf.tile([128, 2, 128], BF16, tag="hT")
            for mch in range(2):
                pff = psf.tile([128, 128], F32, tag="pfx")
                for c in range(2):
                    nc.tensor.matmul(
                        pff[:], FW1[:, l, c, mch * 128:(mch + 1) * 128],
                        xT[:, c, :], start=(c == 0), stop=(c == 1))
                nc.scalar.activation(out=hT[:, mch, :], in_=pff[:],
                                     func=ACTF.Relu,
                                     bias=FB1[:, l, mch:mch + 1], scale=1.0)
            for mch in range(2):
                pff = psf.tile([128, 128], F32, tag="pfx")
                for c in range(2):
                    nc.tensor.matmul(
                        pff[:], FW2[:, l, c, mch * 128:(mch + 1) * 128],
                        hT[:, c, :], start=(c == 0), stop=(c == 1))
                foT = pf.tile([128, 128], F32, tag="foT")
                nc.scalar.activation(out=foT[:], in_=pff[:],
                                     func=ACTF.Identity,
                                     bias=FB2[:, l, mch:mch + 1], scale=1.0)
                ptb = psf.tile([128, 128], F32, tag="pfx")
                nc.tensor.transpose(ptb[:], foT[:], ident[:])
                (radd or nc.vector).tensor_tensor(
                    out=av[t][:, mch * 128:(mch + 1) * 128], in0=ptb[:],
                    in1=av[t][:, mch * 128:(mch + 1) * 128], op=op.add)
            ln_tile(pf, t, 2 * l + 1)
            if dec_xT is not None:
                for c in range(2):
                    ptd = psf.tile([128, 128], BF16, tag="pfx")
                    nc.tensor.transpose(ptd[:], av[t][:, c * 128:(c + 1) * 128],
                                        identb[:])
                    nc.scalar.copy(out=dec_xT[:, c, :], in_=ptd[:])

        # ---------------- main schedule ----------------
        # phase 1: q0 first (so its staging/gather overlaps the MLP), then
        # the layer-0 K/V MLP with a deep W1 psum pipeline (keeps PE ramped);
        # relus split across Act/DVE/Pool
        noh_all = wc.tile([128, NT, RES], BF16)
        patg_cm = tc.tile_pool(name="attg", bufs=2)
        patg = patg_cm.__enter__()
        with (
            tc.tile_pool(name="q0", bufs=2) as pq0,
            tc.tile_pool(name="psq", bufs=1, space="PSUM") as psq,
            tc.tile_pool(name="mlp0", bufs=2) as pm0,
            tc.tile_pool(name="mlph0", bufs=2) as pmh0,
            tc.tile_pool(name="psA1", bufs=3, space="PSUM") as psA1,
            tc.tile_pool(name="psA2", bufs=2, space="PSUM") as psA2,
            tc.tile_pool(name="psA3", bufs=2, space="PSUM") as psA3,
        ):
            # q0 + layer-0 MLP interleaved per chunk (no HOL stall on the
            # XT staging); W3/staging lagged one chunk
            h2s_hist = {}
            for tch in range(V // TCH):
                for vb in range(tch * 4, tch * 4 + 4):
                    vsl = slice(vb * 128, (vb + 1) * 128)
                    pq = psq.tile([128, HD], F32, tag="pq")
                    for c in range(2):
                        nc.tensor.matmul(pq[:], XT[:, c, vsl], WdsC[:, c, :],
                                         start=(c == 0), stop=(c == 1))
                    stq = pq0.tile([128, HD], BF16, tag="stq")
                    nc.scalar.copy(out=stq[:], in_=pq[:])
                    row0 = vb * 128
                    nc.sync.dma_start(
                        out=q0d[row0:row0 + 128, :].rearrange(
                            "(c p) e -> p c e", p=128),
                        in_=stq[:].unsqueeze(1))
                h2s_hist[tch] = {}
                pairs = [(s, h) for s in range(2) for h in range(H)]
                mlp_part12(pm0, pmh0, psA1, psA2, h2s_hist[tch], 0, tch,
                           pairs, (relu_act, relu_act, relu_dve),
                           (relu_dve, relu_act), depth=3, fold_u=True)
                if tch > 0:
                    mlp_part3(pm0, psA3, h2s_hist.pop(tch - 1), 0, tch - 1)
                if tch == 0:
                    # NLL mask precompute on Pool (idle early)
                    with tc.tile_pool(name="nohp", bufs=2) as pn:
                        for t in range(NT):
                            us = pn.tile([128, 1], F32, tag="us")
                            nc.gpsimd.tensor_scalar_mul(
                                out=us[:], in0=u_keep[:, t:t + 1],
                                scalar1=float(RES))
                            us1 = pn.tile([128, 1], F32, tag="us1")
                            nc.gpsimd.tensor_scalar_add(out=us1[:], in0=us[:],
                                                        scalar1=-1.0)
                            A = pn.tile([128, RES], F32, tag="A")
                            nc.gpsimd.tensor_scalar(out=A[:], in0=iota100[:],
                                                    scalar1=us[:], scalar2=None,
                                                    op0=op.is_le, op1=op.bypass)
                            Bm = pn.tile([128, RES], F32, tag="Bm")
                            nc.gpsimd.tensor_scalar(out=Bm[:], in0=iota100[:],
                                                    scalar1=us1[:], scalar2=None,
                                                    op0=op.is_le, op1=op.bypass)
                            nc.gpsimd.tensor_tensor(out=noh_all[:, t, :],
                                                    in0=Bm[:], in1=A[:],
                                                    op=op.subtract)
            # q-gather + av init (q0d staged once chunk 7's q0 block ran;
            # overlaps the final W3/staging)
            with tc.tile_pool(name="qg", bufs=1) as pqg:
                Qg = pqg.tile([128, NT, HD], BF16)
                nc.gpsimd.dma_gather(Qg[:], q0d[:, :], idx_pred[:],
                                     num_idxs=P, num_idxs_reg=P,
                                     elem_size=HD, single_packet=False)
                for t in range(NT):
                    nc.vector.tensor_tensor(out=av[t][:], in0=Qg[:, t, :],
                                            in1=BdsT[:], op=op.add)
            mlp_part3(pm0, psA3, h2s_hist.pop(V // TCH - 1), 0, V // TCH - 1)
            # first attention gathers fire as soon as kv0 staging lands
            Gs = {0: att_gather(patg, 0, 0), 1: att_gather(patg, 0, 1)}

        # ---- unified pipeline: 32 steps = (layer 0 tiles, layer 1 tiles);
        # per-step attention + LN; ff/dec lag 2 steps behind; layer-1 MLP
        # front-loaded into the first 8 steps; gathers prefetched across the
        # layer boundary
        sume_all = wc.tile([128, NT], F32)
        tls_all = wc.tile([128, NT], F32)
        steps = [(0, t) for t in range(NT)] + [(1, t) for t in range(NT)]
        from contextlib import ExitStack
        with (
            tc.tile_pool(name="mlp", bufs=2) as pm,
            tc.tile_pool(name="mlph", bufs=1) as pmh,
            tc.tile_pool(name="att", bufs=1) as pat,
            tc.tile_pool(name="ff", bufs=2) as pf,
            tc.tile_pool(name="psf", bufs=2, space="PSUM") as psf,
            tc.tile_pool(name="dec", bufs=2) as pd,
        ):
            mlp_ps = ExitStack()
            ps1 = mlp_ps.enter_context(
                tc.tile_pool(name="ps1", bufs=2, space="PSUM"))
            ps2 = mlp_ps.enter_context(
                tc.tile_pool(name="ps2", bufs=2, space="PSUM"))
            ps3 = mlp_ps.enter_context(
                tc.tile_pool(name="ps3", bufs=2, space="PSUM"))
            psd_cm = None
            patg2 = None
            issued = 2

            def gpool(k):
                return patg2 if (patg2 is not None and k % 3 == 0) else patg

            h2s_l1 = {}
            pairsA = [(s, h) for s in range(2) for h in range(H // 2)]
            pairsB = [(s, h) for s in range(2) for h in range(H // 2, H)]
            chunk_at = {0: 0, 2: 1, 4: 2, 5: 3, 7: 4, 9: 5, 11: 6, 13: 7}
            LAG = 1
            for sp in range(len(steps) + LAG):
                if sp == 14:
                    mlp_ps.close()  # free MLP psum banks for the decoder
                    psd_cm = tc.tile_pool(name="psd", bufs=2, space="PSUM")
                    psd = psd_cm.__enter__()
                    # MLP inputs/weights done: free their SBUF for a third
                    # gather buffer (deeper prefetch)
                    pxt_cm.__exit__(None, None, None)
                    patg2_cm = tc.tile_pool(name="attg2", bufs=1, side="right")
                    patg2 = patg2_cm.__enter__()
                if sp < len(steps):
                    l, t = steps[sp]
                    want = sp + 1 if (patg2 is None or sp == 14) else sp + 2
                    while issued <= min(want, len(steps) - 1):
                        Gs[issued] = att_gather(gpool(issued), *steps[issued])
                        issued += 1
                    Gcur = Gs.pop(sp)
                    Es, wE = att_score(pat, l, t, Gcur)
                    if sp > 0:
                        lp, tp = steps[sp - 1]
                        ln_tile(pat, tp, 2 * lp)
                    att_value(pat, l, t, Gcur, Es, wE,
                              nsv=8)
                    if sp == len(steps) - 1:
                        ln_tile(pat, t, 2 * l)  # eager LN for the last tile
                    # layer-1 MLP: one chunk per scheduled step
                    if sp in chunk_at:
                        tch = chunk_at[sp]
                        # DVE idles during the first gathers: give it the
                        # first chunk's relus
                        r1 = (relu_dve, relu_act) if tch < 1 else (relu_act,)
                        r2 = (relu_act, relu_dve) if tch < 1 else (relu_act,)
                        mlp_part12(pm, pmh, ps1, ps2, h2s_l1, 1, tch,
                                   pairsA, r1, r2)
                        mlp_part12(pm, pmh, ps1, ps2, h2s_l1, 1, tch,
                                   pairsB, r1, r2)
                        mlp_part3(pm, ps3, h2s_l1, 1, tch)
                if sp < LAG:
                    continue
                ld, td = steps[sp - LAG]
                if ld == 0:
                    ff_tile(pf, psf, 0, td)
                    continue
                xTd = pd.tile([128, 2, 128], BF16, tag="dxT")
                ff_tile(pf, psf, 1, td, dec_xT=xTd)
                # decoder for tile td
                pp = psd.tile([128, 128], F32, tag="pdx")
                for c in range(2):
                    nc.tensor.matmul(pp[:], DW1[:, c, :], xTd[:, c, :],
                                     start=(c == 0), stop=(c == 1))
                h1T = pd.tile([128, 128], BF16, tag="h1T")
                nc.scalar.activation(out=h1T[:], in_=pp[:], func=ACTF.Relu,
                                     bias=DB1[:], scale=1.0)
                pp2 = psd.tile([128, 128], F32, tag="pdx")
                nc.tensor.matmul(pp2[:], DW2[:], h1T[:], start=True, stop=True)
                h2T = pd.tile([128, 128], BF16, tag="h2T")
                nc.scalar.activation(out=h2T[:], in_=pp2[:], func=ACTF.Relu,
                                     bias=DB2[:], scale=1.0)
                pl = psd.tile([128, RES], F32, tag="pdx")
                nc.tensor.matmul(pl[:], h2T[:], DW3[:], start=True, stop=False)
                nc.tensor.matmul(pl[:], ones_row[:], DB3[:], start=False, stop=True)
                escr = pd.tile([128, RES], F32, tag="descr")
                nc.scalar.activation(out=escr[:], in_=pl[:], func=ACTF.Exp,
                                     bias=0.0, scale=1.0,
                                     accum_out=sume_all[:, td:td + 1])
                scr = pd.tile([128, RES], F32, tag="descr")
                nc.vector.tensor_tensor(out=scr[:], in0=pl[:], in1=noh_all[:, td, :],
                                        op=op.mult)
                nc.vector.tensor_reduce(out=tls_all[:, td:td + 1], in_=scr[:],
                                        axis=X, op=op.add)
            if psd_cm is not None:
                psd_cm.__exit__(None, None, None)
            if patg2 is not None:
                patg2_cm.__exit__(None, None, None)
        patg_cm.__exit__(None, None, None)

        # ---------------- final loss reduction ----------------
        with tc.tile_pool(name="fin", bufs=1) as pd:
            # loss = sum_t tls + ln(sume) - log(RES)
            lnall = pd.tile([128, NT], F32, tag="lnall")
            nc.scalar.activation(out=lnall[:], in_=sume_all[:], func=ACTF.Ln,
                                 bias=0.0, scale=1.0)
            nc.vector.tensor_tensor(out=lnall[:], in0=lnall[:], in1=tls_all[:],
                                    op=op.add)
            nc.vector.tensor_reduce(out=loss_acc[:], in_=lnall[:], axis=X, op=op.add)
            nc.vector.tensor_scalar_add(out=loss_acc[:], in0=loss_acc[:],
                                        scalar1=-LOG_RES * NT)
            lsum = pd.tile([128, 1], F32, tag="lsum")
            nc.gpsimd.partition_all_reduce(lsum[:], loss_acc[:], channels=128,
                                           reduce_op=bass_isa.ReduceOp.add)
            nc.gpsimd.dma_start(out=loss_out[:, :], in_=lsum[0:1, :])

        wc_cm.__exit__(None, None, None)

    nc.compile()
    return nc


# PERM[i_new] = old feature index: new order (d,h), old order (h,d)
PERM = np.array([h * D + d for d in range(D) for h in range(H)], dtype=np.int64)


def _pdh(x):
    """Permute last dim from (h,d) order to (d,h) order."""
    return np.ascontiguousarray(x[..., PERM])


_prog_cache = {}
last_exec_time_ns = None
last_trace_path = None


def kernel(**inputs):
    inp = {k: np.asarray(v) for k, v in inputs.items()}
    enc = np.ascontiguousarray(inp["encoded"], dtype=np.float32)
    uu = np.ascontiguousarray(inp["true_u"], dtype=np.float32)
    pred = np.asarray(inp["pred_points"]).astype(np.int64)
    nb = np.asarray(inp["neighbor_index"]).astype(np.int64)

    ln_trivial = all(
        np.all(inp[k] == 1.0) for k in ("ln1_g", "ln2_g")) and all(
        np.all(inp[k] == 0.0) for k in ("ln1_b", "ln2_b"))

    att_list = []
    for t in range(NT):
        idx = np.empty(128 * N, np.int64)
        for n in range(N):
            idx[n * 128:(n + 1) * 128] = nb[t * 128:(t + 1) * 128, n]
        att_list.append(_wrap_idx(idx))
    att_idx = np.concatenate(att_list, axis=1).astype(np.int16)
    pred_idx = _wrap_idx(pred)

    # batched per-head MLP weight layouts: order (l, s, h)
    def stack_ls(kt, vt):
        return np.stack([inp[kt], inp[vt]], axis=1)  # [L, 2, H, ...]

    W1f = stack_ls("kW1", "vW1").astype(np.float32)  # [L,2,H,257,128]
    w1a = np.ascontiguousarray(
        W1f[:, :, :, :256, :].reshape(L, 2, H, 2, 128, MLP)
        .transpose(4, 0, 1, 2, 3, 5).reshape(128, L * 2 * H * 2 * MLP))
    w1u = np.ascontiguousarray(W1f[:, :, :, 256, :].reshape(1, L * 2 * H * MLP))
    w1uc = np.ascontiguousarray(
        W1f[:, :, :, 256, :].reshape(L * 2 * H, MLP).T.reshape(128, L * 2 * H))
    w2a = np.ascontiguousarray(
        stack_ls("kW2", "vW2").astype(np.float32)
        .transpose(3, 0, 1, 2, 4).reshape(128, L * 2 * H * MLP))
    w3a = np.ascontiguousarray(
        stack_ls("kW3", "vW3").astype(np.float32)
        .transpose(3, 0, 1, 2, 4).reshape(128, L * 2 * H * D))
    b1a = np.ascontiguousarray(
        stack_ls("kb1", "vb1").astype(np.float32)
        .transpose(3, 0, 1, 2).reshape(128, L * 2 * H))
    b2a = np.ascontiguousarray(
        stack_ls("kb2", "vb2").astype(np.float32)
        .transpose(3, 0, 1, 2).reshape(128, L * 2 * H))
    # b3 rows in natural (s,h,d) order: [k | v]
    b3shd = np.concatenate([inp["kb3"].reshape(L, HD),
                            inp["vb3"].reshape(L, HD)],
                           axis=1).astype(np.float32)

    fW1p = np.ascontiguousarray(
        inp["fW1"].astype(np.float32)[:, PERM, :][:, :, PERM])
    fW2p = np.ascontiguousarray(
        inp["fW2"].astype(np.float32)[:, PERM, :][:, :, PERM])
    dW1p = np.ascontiguousarray(inp["dW1"].astype(np.float32)[PERM, :])

    ln_g = _pdh(np.stack([inp["ln1_g"][0], inp["ln2_g"][0],
                          inp["ln1_g"][1], inp["ln2_g"][1]]).astype(np.float32))
    ln_b = _pdh(np.stack([inp["ln1_b"][0], inp["ln2_b"][0],
                          inp["ln1_b"][1], inp["ln2_b"][1]]).astype(np.float32))

    shared = {
        "w1a": w1a.astype(BF16NP), "w1u": w1u.astype(BF16NP),
        "w2a": w2a.astype(BF16NP), "w3a": w3a.astype(BF16NP),
        "w1uc": w1uc, "b1a": b1a, "b2a": b2a,
        "b3shd": b3shd,
        "Wds": _pdh(inp["Wds"].astype(np.float32)).astype(BF16NP),
        "bds": _pdh(inp["bds"].reshape(1, HD).astype(np.float32)),
        "fW1": fW1p.astype(BF16NP), "fb1": _pdh(inp["fb1"].astype(np.float32)),
        "fW2": fW2p.astype(BF16NP), "fb2": _pdh(inp["fb2"].astype(np.float32)),
        "dW1": dW1p, "db1": inp["db1"].reshape(MLP, 1).astype(np.float32),
        "dW2": inp["dW2"].astype(np.float32),
        "db2": inp["db2"].reshape(MLP, 1).astype(np.float32),
        "dW3": inp["dW3"].astype(np.float32),
        "db3": inp["db3"].reshape(1, RES).astype(np.float32),
        "ln_g": ln_g, "ln_b": ln_b,
        "att_idx": att_idx, "pred_idx": pred_idx,
    }

    in_maps = []
    for b in range(B):
        m = dict(shared)
        m["enc"] = np.ascontiguousarray(enc[b]).astype(BF16NP)
        m["uu"] = np.ascontiguousarray(uu[b].reshape(V, 1))
        m["u_pred"] = np.ascontiguousarray(
            uu[b][pred].reshape(NT, 128).T.astype(np.float32))
        in_maps.append(m)

    key = ("prog", ln_trivial)
    if key not in _prog_cache:
        _prog_cache[key] = build_program(ln_trivial)
    nc = _prog_cache[key]

    import os
    trace = os.environ.get("BASS_TRACE", "0") == "1"
    res = run_bass_kernel_spmd(nc, in_maps, core_ids=list(range(B)), trace=trace)
    global last_exec_time_ns, last_trace_path
    last_exec_time_ns = res.exec_time_ns
    last_trace_path = res.instructions_and_trace[1] if res.instructions_and_trace else None
    out = np.array([res.results[b]["loss_out"][0, 0] for b in range(B)], dtype=np.float32)
    return out

